# revision 3
# baseline (speedup 1.0000x reference)
"""Trainium2 Bass kernel for a 3-layer GCN binary graph classifier (v2).

Self-contained: takes the FULL inputs of reference.setup_inputs(), shards
across 8 NeuronCores internally, returns the FULL [64,1] output.

v2 design (emulator-cost-aware): the backend charges ~50us per
compute-engine instruction regardless of operand size, while DMA /
dma_gather (<=1024 idxs) / unique-idx dma_scatter_add / collectives are
~free.  So each layer is expressed in ~45 compute instructions:

  - transform: 13 matmuls (512-wide free dim, feature-major) + 4 PSUM
    evacuations (DVE scalar_tensor_tensor folding the dinv[src] scale)
  - M' table: transposing DMA to node-row DRAM + AllGather (free)
  - aggregation: per-dst edge lists padded to power-of-2 rectangles
    (split into lower/upper-half-table slot groups for int16 indices),
    many free 1024-idx dma_gathers, ONE vector.tensor_reduce per chunk
    (reduces the slot axis via a strided 4D AP), then a free unique-idx
    dma_scatter_add into a DRAM accumulator in original node order
  - BN: transposing readback (free), u = dinv[dst]*acc with accum_out
    (1 DVE), sum of squares (1 ACT), tiny stats tail + AllReduce, one
    fused scale/bias+ReLU ACT over the whole [128, 6272] tile
  - pooling: graphs are contiguous node ranges; gather rectangles from
    the node-major h3 table + 9 tensor_reduces + AllReduce; readout is
    a DVE dot (accum_out) + sigmoid ACT (folding 1/cnt and bout).
"""

import dataclasses
import os

import numpy as np

import concourse.bass as bass
import concourse.bacc as bacc
import concourse.mybir as mybir
from concourse.library_config import mlp as _mlp_lib

F32 = mybir.dt.float32
I16 = mybir.dt.int16
AF = mybir.ActivationFunctionType
ALU = mybir.AluOpType

H = 128
EPS = 1e-5
P_CORES = 8
CHUNK_POS = 16384     # gather-buffer positions per chunk ([128, 16384] f32)
CALL_MAX = 1024       # SWDGE ring cap per dma_gather/scatter call


def _next_pow2(x):
    return 1 << int(np.ceil(np.log2(max(int(x), 1))))


def _round_even(x):
    return max(2 * ((int(x) + 1) // 2), 2)


def _pack_idx(arr):
    """[n] int array (n % 16 == 0) -> [128, n//16] int16 (16-wrap, tiled)."""
    w = arr.reshape(-1, 16).T.astype(np.int16)
    return np.tile(w, (8, 1))


# ---------------------------------------------------------------------------
# host prep (graph structure only -- no feature math)
# ---------------------------------------------------------------------------

def _prep(x, edge_index, batch, P, G):
    N, D = x.shape
    assert D == H and N % P == 0
    S = N // P               # 6250 real nodes per core
    SP = ((S + 127) // 128) * 128   # 6272 padded
    NP = P * SP              # 50176 table rows
    HALF = NP // 2           # 25088
    GP = 128                 # padded graph slots

    src = np.asarray(edge_index[0], dtype=np.int64)
    dst = np.asarray(edge_index[1], dtype=np.int64)
    batch = np.asarray(batch, dtype=np.int64)

    deg = np.bincount(dst, minlength=N).astype(np.float32) + 1.0
    dinv = (1.0 / np.sqrt(deg)).astype(np.float32)

    owner = np.arange(N) // S
    rowid = owner * SP + (np.arange(N) - owner * S)   # global table row

    # per-graph node ranges (batch is sorted)
    gcnt = np.bincount(batch, minlength=G).astype(np.int64)
    gstart = np.concatenate([[0], np.cumsum(gcnt)])[:G]
    assert gcnt.max() < CALL_MAX, gcnt.max()
    cinv = (1.0 / np.maximum(gcnt, 1)).astype(np.float32)

    order = np.argsort(dst, kind="stable")
    src_s, dst_s = src[order], dst[order]
    estart = np.searchsorted(dst_s, np.arange(N))
    eend = np.searchsorted(dst_s, np.arange(N) + 1)

    # phase 1: per-core A/B source lists and bucket sizes
    core_lists = []
    for c in range(P):
        lo, hi = c * S, (c + 1) * S
        A_lists, B_lists, Ks = [], [], np.zeros(S, dtype=np.int64)
        for v in range(lo, hi):
            srcs = rowid[src_s[estart[v]:eend[v]]]
            srcs = np.concatenate([srcs, [rowid[v]]])
            a = srcs[srcs < HALF]
            b = srcs[srcs >= HALF] - HALF
            A_lists.append(a)
            B_lists.append(b)
            Ks[v - lo] = _round_even(max(len(a), len(b), 1))
        assert Ks.max() <= 64, Ks.max()
        core_lists.append((A_lists, B_lists, Ks))

    # phase 2: global (SPMD-uniform) bucket structure = max over cores
    all_K = sorted(set().union(*[set(Ks.tolist()) for _, _, Ks in core_lists]))
    NBK_g = {K: ((max(int((Ks == K).sum()) for _, _, Ks in core_lists)
                  + 127) // 128) * 128 for K in all_K}

    per_core = []
    for c in range(P):
        lo, hi = c * S, (c + 1) * S
        A_lists, B_lists, Ks = core_lists[c]
        padA = 0 * SP + S + 10              # zero row, lower half (core 0)
        padB = 4 * SP + S + 10 - HALF       # zero row, upper half (core 4)
        assert padA < HALF

        chunks = []       # (K, nb, gather_calls, scat_idx)
        gidx_blocks = []  # (packed idx array, base_half)
        dump = 0
        for K in all_K:
            nodes = np.nonzero(Ks == K)[0]          # local ids, orig order
            NBK = NBK_g[K]
            NB = min(CALL_MAX, (CHUNK_POS // (2 * K)) // 128 * 128)
            assert NB >= 128 and NB % 128 == 0
            for c0 in range(0, NBK, NB):
                nb = min(NB, NBK - c0)
                nb = ((nb + 127) // 128) * 128
                cn = nodes[c0:c0 + nb]
                # idx grids [K, nb] for A and B
                ga = np.full((K, nb), padA, dtype=np.int64)
                gb = np.full((K, nb), padB, dtype=np.int64)
                for j, v in enumerate(cn):
                    a, b = A_lists[v], B_lists[v]
                    ga[:len(a), j] = a
                    gb[:len(b), j] = b
                calls = []
                for grid, half in ((ga, 0), (gb, 1)):
                    flat = grid.reshape(-1)          # slot-major positions
                    for q0 in range(0, len(flat), CALL_MAX):
                        w = min(CALL_MAX, len(flat) - q0)
                        calls.append((len(gidx_blocks), w))
                        gidx_blocks.append((_pack_idx(flat[q0:q0 + w]), half))
                # scatter idxs: node -> orig local row; pads -> dump rows
                si = np.empty(nb, dtype=np.int64)
                si[:len(cn)] = cn
                npad = nb - len(cn)
                if npad:
                    si[len(cn):] = SP + np.arange(dump, dump + npad) % 2048
                    dump += npad
                # scatter reads rows j from sbuf [j%128, j//128]
                chunks.append((K, nb, calls, _pack_idx(si)))

        # pooling one-hot [128, T*G]: node t*128+p of this core -> graph
        TT = SP // 128
        pool1h = np.zeros((128, TT * G), dtype=np.float32)
        for t in range(TT):
            for p in range(128):
                n = t * 128 + p
                if n < S:
                    pool1h[p, t * G + int(batch[lo + n])] = 1.0

        # flatten idx blocks into one [128, IC] i16 param + offsets
        offs, col = [], 0
        for blk, half in gidx_blocks:
            offs.append((col, half))
            col += blk.shape[1]
        sc_offs = []
        for (K, nb, calls, sidx) in chunks:
            sc_offs.append(col)
            col += sidx.shape[1]
        IC = col
        idxs = np.zeros((128, IC), dtype=np.int16)
        for (blk, half), (o, _) in zip(gidx_blocks, offs):
            idxs[:, o:o + blk.shape[1]] = blk
        for (K, nb, calls, sidx), o in zip(chunks, sc_offs):
            idxs[:, o:o + sidx.shape[1]] = sidx

        # xsT feature-major [128, SP]
        xsT = np.zeros((128, SP), dtype=np.float32)
        xsT[:, :S] = np.asarray(x[lo:hi], np.float32).T
        dinvb = np.zeros((128, SP), dtype=np.float32)
        dinvb[:, :S] = dinv[lo:hi][None, :]

        per_core.append(dict(
            chunks=chunks, pool1h=pool1h, offs=offs,
            sc_offs=sc_offs, IC=IC, idxs=idxs, xsT=xsT, dinvb=dinvb))

    meta = dict(N=N, P=P, S=S, SP=SP, NP=NP, HALF=HALF, G=G, GP=GP,
                cinv=cinv)
    return meta, per_core


def _pack_cpack(meta, pc, params):
    SP, G = meta["SP"], meta["G"]
    cols = {}
    pos = 0

    def add(name, n):
        nonlocal pos
        cols[name] = pos
        pos += n

    add("smalls", 16)      # g0 be0 g1 be1 g2 be2 EPS bout cinv
    add("w0", 128)
    add("w1", 128)
    add("w2", 128)
    add("woutb", 128)
    add("dinvb", SP)
    add("pool1h", (SP // 128) * G)
    CK = pos

    a = np.zeros((128, CK), dtype=np.float32)
    for i in range(3):
        a[:, cols["smalls"] + 2 * i] = np.asarray(params[f"g{i}"], np.float32)
        a[:, cols["smalls"] + 2 * i + 1] = np.asarray(params[f"be{i}"],
                                                      np.float32)
        a[:, cols[f"w{i}"]:cols[f"w{i}"] + 128] = np.asarray(params[f"W{i}"],
                                                             np.float32)
    a[:, cols["smalls"] + 6] = EPS
    a[:, cols["smalls"] + 7] = float(np.asarray(params["bout"],
                                                np.float32)[0])
    a[:G, cols["smalls"] + 8] = meta["cinv"]
    a[:, cols["woutb"]:cols["woutb"] + 128] = np.broadcast_to(
        np.asarray(params["Wout"], np.float32)[:, 0], (128, 128))
    a[:, cols["dinvb"]:cols["dinvb"] + SP] = pc["dinvb"]
    a[:, cols["pool1h"]:cols["pool1h"] + (SP // 128) * G] = pc["pool1h"]
    return a, cols, CK


# ---------------------------------------------------------------------------
# device program
# ---------------------------------------------------------------------------

def build_nc(meta, pc0, cols, CK, reps=1, no_cc=False, no_gather=False, flat_dma=False):
    P, S, SP, NP, HALF, G, GP = (meta[k] for k in
                                 ("P", "S", "SP", "NP", "HALF", "G", "GP"))
    chunks = pc0["chunks"]
    TT = meta["SP"] // 128
    offs, sc_offs, IC = pc0["offs"], pc0["sc_offs"], pc0["IC"]
    ACC = SP + 2048
    MM = (SP + 511) // 512          # 13 transform matmuls per layer
    EG = [(0, 5), (5, 8), (8, MM)]

    nc = bacc.Bacc("TRN2", num_devices=P)
    rg = [list(range(P))]

    cpack_d = nc.declare_dram_parameter("cpack", [128, CK], F32,
                                        isOutput=False)
    xsT_d = nc.declare_dram_parameter("xsT", [128, SP], F32, isOutput=False)
    idxs_d = nc.declare_dram_parameter("idxs", [128, IC], I16, isOutput=False)
    out_d = nc.declare_dram_parameter("out", [G, 1], F32, isOutput=True)

    cc_in = nc.dram_tensor("cc_in", [SP, H], F32)
    mfull = nc.dram_tensor("mfull", [NP, H], F32, addr_space="Shared")
    acc_d = nc.dram_tensor("acc", [ACC, H], F32)
    zrow = nc.dram_tensor("zrow", [ACC, H], F32)
    h3nm = nc.dram_tensor("h3nm", [ACC, H], F32)
    ar_in = nc.dram_tensor("ar_in", [128, 2], F32)
    ar_out = nc.dram_tensor("ar_out", [128, 2], F32, addr_space="Shared")
    ar2_in = nc.dram_tensor("ar2_in", [G, 128], F32)
    ar2_out = nc.dram_tensor("ar2_out", [G, 128], F32, addr_space="Shared")

    import contextlib
    es = contextlib.ExitStack()

    def sb(name, shape, dt=F32):
        return es.enter_context(nc.sbuf_tensor(name, shape, dt))

    def sem(name):
        return es.enter_context(nc.semaphore(name))

    with es, nc.allow_non_contiguous_dma(reason="feature<->node transposes"):
        gbuf = sb("gbuf", [128, CHUNK_POS])
        redo = sb("redo", [128, 2304])
        X = sb("X", [128, SP])
        Y = sb("Y", [128, SP])
        dinvb = sb("dinvb", [128, SP])
        wsb = sb("wsb", [128, 3 * 128])
        woutb = sb("woutb", [128, 128])
        smalls = sb("smalls", [128, 16])
        idxs_sb = sb("idxs_sb", [128, IC], I16)
        stp = sb("stp", [128, 2])
        sta = sb("sta", [128, 2])
        bnp = sb("bnp", [128, 8])
        parb = sb("parb", [G, 128])
        p1h = sb("p1h", [128, TT * 64])
        dota = sb("dota", [128, 1])
        outs = sb("outs", [G, 1])
        psT = es.enter_context(nc.psum_tensor("psT", [128, 8, 512], F32))

        s_dma = sem("s_dma")    # all sync-engine DMA completions (+16)
        s_g4 = [sem(f"s_g{i}") for i in range(4)]  # gather sems (+16)
        s_sc = sem("s_sc")      # scatter completions (+16)
        s_cc = sem("s_cc")      # collectives (+1)
        s_z = sem("s_z")        # zrow memset (vector, +1)
        s_pet = sem("s_pet")    # matmuls (+1)
        s_ev = sem("s_ev")      # psum evac groups (+1)
        s_red = sem("s_red")    # reduces (+1)
        s_u = sem("s_u")        # u-stt done (+1)
        s_sq = sem("s_sq")      # square done (+1)
        s_b1 = sem("s_b1")
        s_b2 = sem("s_b2")
        s_b3 = sem("s_b3")
        s_h = sem("s_h")        # BN+ReLU done (+1)
        s_dot = sem("s_dot")    # dot stt done (+1)
        s_sig = sem("s_sig")
        s_pool = sem("s_pool")
        s_pp = sem("s_pp")    # sigmoid done (+1)

        # python-side tallies for DMA-completion waits
        T = {"dma": 0, "g": 0, "sc": 0, "cc": 0}
        M = {}                   # named checkpoints -> tally value

        def ap4(base_ap, dims):
            return dataclasses.replace(base_ap, ap=[list(base_ap.ap[0])] +
                                       [list(d) for d in dims])

        def col(name, o=0, n=1):
            return smalls[:, cols_smalls[name] + o:cols_smalls[name] + o + n]

        cols_smalls = {"g0": 0, "be0": 1, "g1": 2, "be1": 3, "g2": 4,
                       "be2": 5, "eps": 6, "bout": 7, "cinv": 8}
        for k in list(cols_smalls):
            cols_smalls[k] = cols_smalls[k]

        HB = [X, Y]

        with nc.Block() as block:

            @block.sync
            def _(sync):
                def dma(out, in_, name=None):
                    # serialize: completions ordered => threshold waits sound
                    sync.wait_ge(s_dma, T["dma"])
                    T["dma"] += 16
                    if name:
                        M[name] = T["dma"]
                    sync.dma_start(out=out, in_=in_).then_inc(s_dma, 16)

                dma(idxs_sb[:, :], idxs_d[:, :])
                dma(dinvb[:, :], cpack_d[:, cols["dinvb"]:cols["dinvb"] + SP])
                dma(wsb[:, :], cpack_d[:, cols["w0"]:cols["w0"] + 3 * 128])
                dma(woutb[:, :], cpack_d[:, cols["woutb"]:cols["woutb"] + 128])
                dma(smalls[:, :], cpack_d[:, cols["smalls"]:cols["smalls"] + 16],
                    "smalls")
                dma(p1h[:, :], cpack_d[:, cols["pool1h"]:
                                       cols["pool1h"] + TT * 64])
                # zrow from memset gbuf
                sync.wait_ge(s_z, 1)
                zs = ACC // 128
                dma(zrow.ap().rearrange("(p a) f -> p (a f)", p=128),
                    gbuf[:, 0:zs * 128], "zrow")
                # scrub internal DRAM so runs are independent of device
                # history (stale NaNs from unrelated programs must not be
                # observable through any read-early window)
                dma(acc_d[:, :], zrow[:, :])
                dma(h3nm[:, :], zrow[:, :])
                dma(cc_in[:, :], zrow[0:SP, :])
                for zi in range(NP // ACC):
                    dma(mfull[zi * ACC:(zi + 1) * ACC, :], zrow[:, :])
                dma(mfull[NP - ACC:NP, :], zrow[:, :])
                dma(ar_in[:, :], zrow[0:128, 0:2])
                dma(ar_out[:, :], zrow[0:128, 0:2])
                dma(ar2_in[:, :], zrow[0:G, :])
                dma(ar2_out[:, :], zrow[0:G, :], "scrub")
                for rep in range(reps):
                    dma(X[:, :], xsT_d[:, :], f"x{rep}")
                    for l in range(3):
                        LG = rep * 3 + l
                        Hs, Ms = HB[l % 2], HB[(l + 1) % 2]
                        # M' store (transposing) after all evacs of layer
                        sync.wait_ge(s_ev, LG * len(EG) + len(EG))
                        if flat_dma:
                            dma(cc_in.ap().rearrange("(p t) f -> p (t f)",
                                                     p=128),
                                Ms[:, :], f"st{LG}")
                        else:
                            dma(cc_in.ap().rearrange("(n o) f -> f (n o)",
                                                     o=1),
                                Ms[:, :], f"st{LG}")
                        # zero acc (before scatters; after prior readback)
                        dma(acc_d[:, :], zrow[:, :], f"z{LG}")
                        # readback acc -> Ms (transposing) after scatters
                        sync.wait_ge(s_sc, 16 * sum(
                            1 for _ in _iter_scat(chunks, LG + 1)))
                        if flat_dma:
                            dma(Ms.ap(), acc_d.ap().rearrange(
                                "(p t) f -> p (t f)", p=128)[:, 0:SP],
                                f"rb{LG}")
                        else:
                            dma(Ms.ap(),
                                dataclasses.replace(
                                    acc_d.ap(),
                                    ap=[[1, 128], [H, SP]]),
                                f"rb{LG}")
                        # stats store / load around AllReduce
                        sync.wait_ge(s_u, LG + 1)
                        sync.wait_ge(s_sq, LG + 1)
                        dma(ar_in[:, :], stp[:, :], f"ss{LG}")
                        sync.wait_ge(s_cc, (rep * 7 + 2 * l + 2) *
                                     (16 if no_cc else 1))
                        dma(sta[:, :], ar_out[:, :], f"sl{LG}")
                    # pooling: transposing store of h3 (Y) to node rows
                    sync.wait_ge(s_h, rep * 3 + 3)
                    if flat_dma:
                        dma(h3nm.ap().rearrange("(p t) f -> p (t f)",
                                                p=128)[:, 0:SP],
                            HB[1][:, :], f"h3{rep}")
                    else:
                        dma(h3nm.ap().rearrange("(n o) f -> f (n o)", o=1)[
                            :, 0:SP], HB[1][:, :], f"h3{rep}")
                    # node-major reload: sbuf[p, t, f] = h3nm[t*128+p, f]
                    dma(gbuf[:, 0:TT * 128].rearrange(
                            "p (t f) -> p t f", f=128),
                        dataclasses.replace(
                            h3nm.ap(),
                            ap=[[H, 128], [128 * H, TT], [1, H]]),
                        f"pm{rep}")
                    # pooled partial AllReduce
                    sync.wait_ge(s_pp, rep + 1)
                    dma(ar2_in.ap().rearrange("(p o) f -> p (o f)", p=G),
                        parb[:, :], f"ps{rep}")
                    sync.wait_ge(s_cc, (rep * 7 + 7) * (16 if no_cc else 1))
                    dma(parb[:, :],
                        ar2_out.ap().rearrange("(p o) f -> p (o f)", p=G),
                        f"pl{rep}")
                    sync.wait_ge(s_sig, rep + 1)
                    dma(out_d[:, :], outs[:, :])

            def _cc(gpsimd, kind, op, ins, outs_):
                if no_cc:
                    T["cc"] += 16
                    return gpsimd.dma_start(
                        out=outs_[0].tensor[0:ins[0].shape[0], :],
                        in_=ins[0]).then_inc(s_cc, 16)
                T["cc"] += 1
                return gpsimd.collective_compute(
                    kind, op, replica_groups=rg, ins=ins,
                    outs=outs_).then_inc(s_cc, 1)

            @block.gpsimd
            def _(gpsimd):
                gpsimd.load_library(_mlp_lib)
                red_n = 0
                for rep in range(reps):
                    for l in range(3):
                        LG = rep * 3 + l
                        gpsimd.wait_ge(s_dma, M[f"st{LG}"])
                        _cc(gpsimd, "AllGather", ALU.bypass,
                            [cc_in[:, :]], [mfull[:, :]])
                        gpsimd.wait_ge(s_cc, T["cc"])
                        gpsimd.wait_ge(s_dma, M[f"z{LG}"])
                        if l == 0 and rep > 0:
                            # pool matmuls of rep-1 still read gbuf
                            gpsimd.wait_ge(s_pool, rep)
                        gsl = LG % 4
                        for (K, nb, calls, _sidx), so in zip(chunks, sc_offs):
                            # gathers overwrite gbuf: wait prior reduce
                            gpsimd.wait_ge(s_red, red_n)
                            off = 0
                            for (bi, w) in calls:
                                if no_gather:
                                    continue
                                o, half = offs[bi]
                                base = mfull[0:HALF, :] if half == 0 else \
                                    mfull[HALF:NP, :]
                                T["g" + str(gsl)] = T.get("g" + str(gsl),
                                                          0) + 16
                                gpsimd.dma_gather(
                                    gbuf[:, off:off + w].rearrange(
                                        "p (c e) -> p c e", e=128),
                                    base, idxs_sb[:, o:o + w // 16],
                                    w, w, 128,
                                ).then_inc(s_g4[gsl], 16)
                                off += w
                            red_n += 1
                            gpsimd.wait_ge(s_red, red_n)
                            T["sc"] += 16
                            gpsimd.dma_scatter_add(
                                acc_d[:, :],
                                redo[:, 0:nb].rearrange(
                                    "p (c e) -> p c e", e=128),
                                idxs_sb[:, so:so + nb // 16],
                                nb, nb, 128,
                            ).then_inc(s_sc, 16)
                        gpsimd.wait_ge(s_dma, M[f"ss{LG}"])
                        _cc(gpsimd, "AllReduce", ALU.add,
                            [ar_in[:, :]], [ar_out[:, :]])
                    # pooled partial AllReduce
                    gpsimd.wait_ge(s_dma, M[f"ps{rep}"])
                    _cc(gpsimd, "AllReduce", ALU.add,
                        [ar2_in[:, :]], [ar2_out[:, :]])

            @block.tensor
            def _(tensor):
                for rep in range(reps):
                    for l in range(3):
                        LG = rep * 3 + l
                        Hs = HB[l % 2]
                        for m in range(MM):
                            if m == 0:
                                if l == 0:
                                    tensor.wait_ge(s_dma, M[f"x{rep}"])
                                    tensor.wait_ge(s_dma, M["smalls"])
                                else:
                                    tensor.wait_ge(s_h, LG)
                            if m == 8:
                                tensor.wait_ge(s_ev, LG * len(EG) + 1)
                            w = min(512, SP - m * 512)
                            tensor.matmul(
                                psT[:, m % 8, 0:w],
                                wsb[:, l * 128:(l + 1) * 128],
                                Hs[:, m * 512:m * 512 + w],
                            ).then_inc(s_pet, 1)
                    # pooling: pooled[g, f] = sum_t onehot_t.T @ h3_t
                    tensor.wait_ge(s_dma, M[f"pm{rep}"])
                    for t in range(TT):
                        tensor.matmul(
                            psT[0:G, 5, 0:128],
                            p1h[:, t * 64:t * 64 + 64][:, 0:G],
                            gbuf[:, 0:TT * 128].rearrange(
                                "p (t2 f) -> p t2 f", f=128)[:, t, :],
                            start=(t == 0), stop=(t == TT - 1),
                        )
                    tensor.drain()
                    tensor.matmul(
                        psT[0:1, 6, 0:1], p1h[0:1, 0:1], p1h[0:1, 0:1],
                    ).then_inc(s_pool, 1)

            @block.vector
            def _(vector):
                vector.memset(gbuf[:, 0:(ACC // 128) * 128], 0.0).then_inc(
                    s_z, 1)
                g_n = [0, 0, 0, 0]
                sc_n = 0
                red_n = 0
                for rep in range(reps):
                    for l in range(3):
                        LG = rep * 3 + l
                        Ms = HB[(l + 1) % 2]
                        Hs = HB[l % 2]
                        # psum evacuations with dinv fold
                        for gi, (m0, m1) in enumerate(EG):
                            vector.wait_ge(s_pet, LG * MM + m1)
                            c0, c1 = m0 * 512, min(m1 * 512, SP)
                            vector.scalar_tensor_tensor(
                                Ms[:, c0:c1],
                                psT[:, :, :].rearrange(
                                    "p a b -> p (a b)")[:, c0 - m0 * 512 +
                                                        (m0 % 8) * 512:
                                                        (m0 % 8) * 512 +
                                                        c1 - m0 * 512],
                                1.0,
                                dinvb[:, c0:c1],
                                op0=ALU.mult, op1=ALU.mult,
                            ).then_inc(s_ev, 1)
                        # chunk reduces
                        gsl = LG % 4
                        for (K, nb, calls, _sidx) in chunks:
                            if not no_gather:
                                g_n[gsl] += 16 * len(calls)
                            vector.wait_ge(s_g4[gsl], g_n[gsl])
                            if sc_n:
                                vector.wait_ge(s_sc, sc_n)
                            inap = ap4(gbuf[:, 0:128],
                                       [[128, nb // 128], [1, 128],
                                        [nb, 2 * K]])
                            outap = ap4(redo[:, 0:128],
                                        [[128, nb // 128], [1, 128]])
                            vector.tensor_reduce(
                                outap, inap, mybir.AxisListType.X, ALU.add,
                            ).then_inc(s_red, 1)
                            red_n += 1
                            sc_n += 16
                        # u = dinv * acc with accum
                        vector.wait_ge(s_dma, M[f"rb{LG}"])
                        vector.scalar_tensor_tensor(
                            Hs[:, :], Ms[:, :], 1.0, dinvb[:, :],
                            op0=ALU.mult, op1=ALU.mult,
                            accum_out=stp[:, 0:1],
                        ).then_inc(s_u, 1)
                        # BN stats tail
                        vector.wait_ge(s_dma, M[f"sl{LG}"])
                        vector.tensor_scalar_mul(bnp[:, 0:2], sta[:, 0:2],
                                                 1.0 / meta["N"])
                        vector.drain()
                        vector.tensor_mul(bnp[:, 2:3], bnp[:, 0:1],
                                          bnp[:, 0:1])
                        vector.drain()
                        vector.tensor_sub(bnp[:, 2:3], bnp[:, 1:2],
                                          bnp[:, 2:3])
                        vector.drain()
                        vector.tensor_scalar_add(bnp[:, 2:3], bnp[:, 2:3],
                                                 EPS)
                        vector.drain()
                        vector.reciprocal(bnp[:, 4:5],
                                          bnp[:, 2:3]).then_inc(s_b1, 1)
                        vector.wait_ge(s_b2, LG + 1)
                        vector.tensor_sub(bnp[:, 3:4],
                                          col(f"be{l}"),
                                          bnp[:, 7:8]).then_inc(s_b3, 1)
                    # pooled partial psum -> sbuf
                    vector.wait_ge(s_pool, rep + 1)
                    vector.tensor_copy(parb[:, :],
                                       psT[0:G, 5, 0:128]).then_inc(s_pp, 1)
                    # readout dot
                    vector.wait_ge(s_dma, M[f"pl{rep}"])
                    vector.scalar_tensor_tensor(
                        redo[0:G, 1024:1024 + 128], parb[:, :], 1.0,
                        woutb[0:G, :], op0=ALU.mult, op1=ALU.mult,
                        accum_out=dota[0:G, 0:1],
                    ).then_inc(s_dot, 1)

            @block.scalar
            def _(scalar):
                for rep in range(reps):
                    for l in range(3):
                        LG = rep * 3 + l
                        Hs = HB[l % 2]
                        Ms = HB[(l + 1) % 2]
                        scalar.wait_ge(s_u, LG + 1)
                        scalar.activation(
                            gbuf[:, 0:SP], Hs[:, :], AF.Square,
                            accum_out=stp[:, 1:2],
                        ).then_inc(s_sq, 1)
                        scalar.wait_ge(s_b1, LG + 1)
                        scalar.activation(bnp[:, 5:6], bnp[:, 4:5], AF.Sqrt)
                        scalar.drain()
                        scalar.activation(bnp[:, 6:7], bnp[:, 5:6], AF.Copy,
                                          scale=col(f"g{l}"))
                        scalar.drain()
                        scalar.activation(bnp[:, 7:8], bnp[:, 6:7], AF.Copy,
                                          scale=bnp[:, 0:1]).then_inc(s_b2, 1)
                        scalar.wait_ge(s_b3, LG + 1)
                        scalar.activation(
                            Ms[:, :], Hs[:, :], AF.Relu,
                            bias=bnp[:, 3:4], scale=bnp[:, 6:7],
                        ).then_inc(s_h, 1)
                    scalar.wait_ge(s_dot, rep + 1)
                    scalar.activation(
                        outs[:, :], dota[0:G, 0:1], AF.Sigmoid,
                        bias=col("bout").tensor[0:G,
                                                cols_smalls["bout"]:
                                                cols_smalls["bout"] + 1],
                        scale=col("cinv").tensor[0:G,
                                                 cols_smalls["cinv"]:
                                                 cols_smalls["cinv"] + 1],
                    ).then_inc(s_sig, 1)

        nc.compile()
    return nc


def _iter_scat(chunks, nlayers):
    for _ in range(nlayers):
        for ch in chunks:
            yield ch


def _redtotal(chunks, nlayers):
    return nlayers * len(chunks)


# ---------------------------------------------------------------------------
# entry point
# ---------------------------------------------------------------------------

def kernel(**inputs):
    x = np.asarray(inputs["x"], np.float32)
    edge_index = np.asarray(inputs["edge_index"])
    batch = np.asarray(inputs["batch"])
    G = 64
    P = P_CORES

    meta, per_core = _prep(x, edge_index, batch, P, G)
    in_maps = []
    cols = CK = None
    for c in range(P):
        cpack, cols, CK = _pack_cpack(meta, per_core[c], inputs)
        in_maps.append({
            "cpack": cpack,
            "xsT": per_core[c]["xsT"],
            "idxs": per_core[c]["idxs"],
        })

    nc = build_nc(meta, per_core[0], cols, CK, reps=1)

    if os.environ.get("GCN_SIM"):
        from concourse import bass_interp
        sim = bass_interp.MultiCoreSim(nc, P)
        for c in range(P):
            for k, v in in_maps[c].items():
                sim.cores[c].tensor(k)[:] = v
        sim.simulate()
        return np.asarray(sim.cores[0].mem_tensor("out"), np.float32)

    from concourse.bass_utils import run_bass_kernel_spmd
    res = run_bass_kernel_spmd(nc, in_maps, core_ids=list(range(P)))
    return np.asarray(res.results[0]["out"], np.float32)


# revision 4
# speedup vs baseline: 1.5116x; 1.5116x over previous
"""Trainium2 Bass kernel for a 3-layer GCN binary graph classifier (v2).

Self-contained: takes the FULL inputs of reference.setup_inputs(), shards
across 8 NeuronCores internally, returns the FULL [64,1] output.

v2 design (emulator-cost-aware): the backend charges ~50us per
compute-engine instruction regardless of operand size, while DMA /
dma_gather (<=1024 idxs) / unique-idx dma_scatter_add / collectives are
~free.  So each layer is expressed in ~45 compute instructions:

  - transform: 13 matmuls (512-wide free dim, feature-major) + 4 PSUM
    evacuations (DVE scalar_tensor_tensor folding the dinv[src] scale)
  - M' table: transposing DMA to node-row DRAM + AllGather (free)
  - aggregation: per-dst edge lists padded to power-of-2 rectangles
    (split into lower/upper-half-table slot groups for int16 indices),
    many free 1024-idx dma_gathers, ONE vector.tensor_reduce per chunk
    (reduces the slot axis via a strided 4D AP), then a free unique-idx
    dma_scatter_add into a DRAM accumulator in original node order
  - BN: transposing readback (free), u = dinv[dst]*acc with accum_out
    (1 DVE), sum of squares (1 ACT), tiny stats tail + AllReduce, one
    fused scale/bias+ReLU ACT over the whole [128, 6272] tile
  - pooling: graphs are contiguous node ranges; gather rectangles from
    the node-major h3 table + 9 tensor_reduces + AllReduce; readout is
    a DVE dot (accum_out) + sigmoid ACT (folding 1/cnt and bout).
"""

import dataclasses
import os

import numpy as np

import concourse.bass as bass
import concourse.bacc as bacc
import concourse.mybir as mybir
from concourse.library_config import mlp as _mlp_lib

F32 = mybir.dt.float32
I16 = mybir.dt.int16
AF = mybir.ActivationFunctionType
ALU = mybir.AluOpType

H = 128
EPS = 1e-5
P_CORES = 8
CHUNK_POS = 16384     # gather-buffer positions per chunk ([128, 16384] f32)
CALL_MAX = 1024       # SWDGE ring cap per dma_gather/scatter call


def _next_pow2(x):
    return 1 << int(np.ceil(np.log2(max(int(x), 1))))


def _round_even(x):
    return max(2 * ((int(x) + 1) // 2), 2)


def _pack_idx(arr):
    """[n] int array (n % 16 == 0) -> [128, n//16] int16 (16-wrap, tiled)."""
    w = arr.reshape(-1, 16).T.astype(np.int16)
    return np.tile(w, (8, 1))


# ---------------------------------------------------------------------------
# host prep (graph structure only -- no feature math)
# ---------------------------------------------------------------------------

def _prep(x, edge_index, batch, P, G):
    N, D = x.shape
    assert D == H and N % P == 0
    S = N // P               # 6250 real nodes per core
    SP = ((S + 127) // 128) * 128   # 6272 padded
    NP = P * SP              # 50176 table rows
    HALF = NP // 2           # 25088
    GP = 128                 # padded graph slots

    src = np.asarray(edge_index[0], dtype=np.int64)
    dst = np.asarray(edge_index[1], dtype=np.int64)
    batch = np.asarray(batch, dtype=np.int64)

    deg = np.bincount(dst, minlength=N).astype(np.float32) + 1.0
    dinv = (1.0 / np.sqrt(deg)).astype(np.float32)

    owner = np.arange(N) // S
    rowid = owner * SP + (np.arange(N) - owner * S)   # global table row

    # per-graph node ranges (batch is sorted)
    gcnt = np.bincount(batch, minlength=G).astype(np.int64)
    gstart = np.concatenate([[0], np.cumsum(gcnt)])[:G]
    assert gcnt.max() < CALL_MAX, gcnt.max()
    cinv = (1.0 / np.maximum(gcnt, 1)).astype(np.float32)

    order = np.argsort(dst, kind="stable")
    src_s, dst_s = src[order], dst[order]
    estart = np.searchsorted(dst_s, np.arange(N))
    eend = np.searchsorted(dst_s, np.arange(N) + 1)

    # phase 1: per-core A/B source lists and bucket sizes
    core_lists = []
    for c in range(P):
        lo, hi = c * S, (c + 1) * S
        A_lists, B_lists, Ks = [], [], np.zeros(S, dtype=np.int64)
        for v in range(lo, hi):
            srcs = rowid[src_s[estart[v]:eend[v]]]
            srcs = np.concatenate([srcs, [rowid[v]]])
            a = srcs[srcs < HALF]
            b = srcs[srcs >= HALF] - HALF
            A_lists.append(a)
            B_lists.append(b)
            Ks[v - lo] = max(len(a), len(b), 1)
        assert Ks.max() <= 64, Ks.max()
        core_lists.append((A_lists, B_lists, Ks))

    # phase 2: global (SPMD-uniform) bucket structure.  Merge adjacent
    # K-buckets until each holds >=256 nodes (worst core) so the
    # 128-node padding and per-bucket chunk overheads stay small.
    raw_K = sorted(set().union(*[set(Ks.tolist()) for _, _, Ks in core_lists]))
    maxcnt = {K: max(int((Ks == K).sum()) for _, _, Ks in core_lists)
              for K in raw_K}
    remap, group, gcnt2 = {}, [], 0
    for K in raw_K:
        group.append(K)
        gcnt2 += maxcnt[K]
        if gcnt2 >= 256:
            for k2 in group:
                remap[k2] = K
            group, gcnt2 = [], 0
    for k2 in group:          # leftover tail -> largest K
        remap[k2] = raw_K[-1]
    for c in range(P):
        A_l, B_l, Ks = core_lists[c]
        core_lists[c] = (A_l, B_l,
                         np.array([remap[int(k)] for k in Ks]))
    all_K = sorted(set(remap.values()))
    NBK_g = {K: ((max(int((Ks == K).sum()) for _, _, Ks in core_lists)
                  + 127) // 128) * 128 for K in all_K}

    per_core = []
    for c in range(P):
        lo, hi = c * S, (c + 1) * S
        A_lists, B_lists, Ks = core_lists[c]
        padA = 0 * SP + S + 10              # zero row, lower half (core 0)
        padB = 4 * SP + S + 10 - HALF       # zero row, upper half (core 4)
        assert padA < HALF

        chunks = []       # (K, nb, gather_calls, scat_idx)
        gidx_blocks = []  # (packed idx array, base_half)
        dump = 0
        for K in all_K:
            nodes = np.nonzero(Ks == K)[0]          # local ids, orig order
            NBK = NBK_g[K]
            NB = min(CALL_MAX, (CHUNK_POS // (2 * K)) // 128 * 128)
            assert NB >= 128 and NB % 128 == 0
            for c0 in range(0, NBK, NB):
                nb = min(NB, NBK - c0)
                nb = ((nb + 127) // 128) * 128
                cn = nodes[c0:c0 + nb]
                # idx grids [K, nb] for A and B
                ga = np.full((K, nb), padA, dtype=np.int64)
                gb = np.full((K, nb), padB, dtype=np.int64)
                for j, v in enumerate(cn):
                    a, b = A_lists[v], B_lists[v]
                    ga[:len(a), j] = a
                    gb[:len(b), j] = b
                calls = []
                for grid, half in ((ga, 0), (gb, 1)):
                    flat = grid.reshape(-1)          # slot-major positions
                    for q0 in range(0, len(flat), CALL_MAX):
                        w = min(CALL_MAX, len(flat) - q0)
                        calls.append((len(gidx_blocks), w))
                        gidx_blocks.append((_pack_idx(flat[q0:q0 + w]), half))
                # scatter idxs: node -> orig local row; pads -> dump rows
                si = np.empty(nb, dtype=np.int64)
                si[:len(cn)] = cn
                npad = nb - len(cn)
                if npad:
                    si[len(cn):] = SP + np.arange(dump, dump + npad) % 2048
                    dump += npad
                # scatter reads rows j from sbuf [j%128, j//128]
                chunks.append((K, nb, calls, _pack_idx(si)))

        # pooling one-hot [128, T*G]: node t*128+p of this core -> graph
        TT = SP // 128
        pool1h = np.zeros((128, TT * G), dtype=np.float32)
        for t in range(TT):
            for p in range(128):
                n = t * 128 + p
                if n < S:
                    pool1h[p, t * G + int(batch[lo + n])] = 1.0

        # flatten idx blocks into one [128, IC] i16 param + offsets
        offs, col = [], 0
        for blk, half in gidx_blocks:
            offs.append((col, half))
            col += blk.shape[1]
        sc_offs = []
        for (K, nb, calls, sidx) in chunks:
            sc_offs.append(col)
            col += sidx.shape[1]
        IC = col
        idxs = np.zeros((128, IC), dtype=np.int16)
        for (blk, half), (o, _) in zip(gidx_blocks, offs):
            idxs[:, o:o + blk.shape[1]] = blk
        for (K, nb, calls, sidx), o in zip(chunks, sc_offs):
            idxs[:, o:o + sidx.shape[1]] = sidx

        # xsT feature-major [128, SP]
        xsT = np.zeros((128, SP), dtype=np.float32)
        xsT[:, :S] = np.asarray(x[lo:hi], np.float32).T
        dinvb = np.zeros((128, SP), dtype=np.float32)
        dinvb[:, :S] = dinv[lo:hi][None, :]

        per_core.append(dict(
            chunks=chunks, pool1h=pool1h, offs=offs,
            sc_offs=sc_offs, IC=IC, idxs=idxs, xsT=xsT, dinvb=dinvb))

    meta = dict(N=N, P=P, S=S, SP=SP, NP=NP, HALF=HALF, G=G, GP=GP,
                cinv=cinv)
    return meta, per_core


def _pack_cpack(meta, pc, params):
    SP, G = meta["SP"], meta["G"]
    cols = {}
    pos = 0

    def add(name, n):
        nonlocal pos
        cols[name] = pos
        pos += n

    add("smalls", 16)      # g0 be0 g1 be1 g2 be2 EPS bout cinv
    add("w0", 128)
    add("w1", 128)
    add("w2", 128)
    add("woutb", 128)
    add("dinvb", SP)
    add("pool1h", (SP // 128) * G)
    CK = pos

    a = np.zeros((128, CK), dtype=np.float32)
    for i in range(3):
        a[:, cols["smalls"] + 2 * i] = np.asarray(params[f"g{i}"], np.float32)
        a[:, cols["smalls"] + 2 * i + 1] = np.asarray(params[f"be{i}"],
                                                      np.float32)
        a[:, cols[f"w{i}"]:cols[f"w{i}"] + 128] = np.asarray(params[f"W{i}"],
                                                             np.float32)
    a[:, cols["smalls"] + 6] = EPS
    a[:, cols["smalls"] + 7] = float(np.asarray(params["bout"],
                                                np.float32)[0])
    a[:G, cols["smalls"] + 8] = meta["cinv"]
    a[:, cols["woutb"]:cols["woutb"] + 128] = np.broadcast_to(
        np.asarray(params["Wout"], np.float32)[:, 0], (128, 128))
    a[:, cols["dinvb"]:cols["dinvb"] + SP] = pc["dinvb"]
    a[:, cols["pool1h"]:cols["pool1h"] + (SP // 128) * G] = pc["pool1h"]
    return a, cols, CK


# ---------------------------------------------------------------------------
# device program
# ---------------------------------------------------------------------------

def build_nc(meta, pc0, cols, CK, reps=1, no_cc=False, no_gather=False, flat_dma=False):
    P, S, SP, NP, HALF, G, GP = (meta[k] for k in
                                 ("P", "S", "SP", "NP", "HALF", "G", "GP"))
    chunks = pc0["chunks"]
    TT = meta["SP"] // 128
    offs, sc_offs, IC = pc0["offs"], pc0["sc_offs"], pc0["IC"]
    ACC = SP + 2048
    MM = (SP + 511) // 512          # 13 transform matmuls per layer
    EG = [(0, 5), (5, 8), (8, MM)]

    nc = bacc.Bacc("TRN2", num_devices=P)
    rg = [list(range(P))]

    cpack_d = nc.declare_dram_parameter("cpack", [128, CK], F32,
                                        isOutput=False)
    xsT_d = nc.declare_dram_parameter("xsT", [128, SP], F32, isOutput=False)
    idxs_d = nc.declare_dram_parameter("idxs", [128, IC], I16, isOutput=False)
    out_d = nc.declare_dram_parameter("out", [G, 1], F32, isOutput=True)

    cc_in = nc.dram_tensor("cc_in", [SP, H], F32)
    mfull = nc.dram_tensor("mfull", [NP, H], F32, addr_space="Shared")
    acc_d = nc.dram_tensor("acc", [ACC, H], F32)
    zrow = nc.dram_tensor("zrow", [ACC, H], F32)
    h3nm = nc.dram_tensor("h3nm", [ACC, H], F32)
    ar_in = nc.dram_tensor("ar_in", [128, 2], F32)
    ar_out = nc.dram_tensor("ar_out", [128, 2], F32, addr_space="Shared")
    ar2_in = nc.dram_tensor("ar2_in", [G, 128], F32)
    ar2_out = nc.dram_tensor("ar2_out", [G, 128], F32, addr_space="Shared")

    import contextlib
    es = contextlib.ExitStack()

    def sb(name, shape, dt=F32):
        return es.enter_context(nc.sbuf_tensor(name, shape, dt))

    def sem(name):
        return es.enter_context(nc.semaphore(name))

    with es, nc.allow_non_contiguous_dma(reason="feature<->node transposes"):
        gbuf = sb("gbuf", [128, CHUNK_POS])
        redo = sb("redo", [128, 2304])
        X = sb("X", [128, SP])
        Y = sb("Y", [128, SP])
        dinvb = sb("dinvb", [128, SP])
        wsb = sb("wsb", [128, 3 * 128])
        woutb = sb("woutb", [128, 128])
        smalls = sb("smalls", [128, 16])
        idxs_sb = sb("idxs_sb", [128, IC], I16)
        stp = sb("stp", [128, 2])
        sta = sb("sta", [128, 2])
        bnp = sb("bnp", [128, 8])
        parb = sb("parb", [G, 128])
        p1h = sb("p1h", [128, TT * 64])
        dota = sb("dota", [128, 1])
        outs = sb("outs", [G, 1])
        psT = es.enter_context(nc.psum_tensor("psT", [128, 8, 512], F32))

        s_dma = sem("s_dma")    # all sync-engine DMA completions (+16)
        s_g4 = [sem(f"s_g{i}") for i in range(4)]  # gather sems (+16)
        s_sc = sem("s_sc")      # scatter completions (+16)
        s_cc = sem("s_cc")      # collectives (+1)
        s_z = sem("s_z")        # zrow memset (vector, +1)
        s_pet = sem("s_pet")    # matmuls (+1)
        s_ev = sem("s_ev")      # psum evac groups (+1)
        s_red = sem("s_red")    # reduces (+1)
        s_u = sem("s_u")        # u-stt done (+1)
        s_sq = sem("s_sq")      # square done (+1)
        s_b1 = sem("s_b1")
        s_b2 = sem("s_b2")
        s_b3 = sem("s_b3")
        s_h = sem("s_h")        # BN+ReLU done (+1)
        s_dot = sem("s_dot")    # dot stt done (+1)
        s_sig = sem("s_sig")
        s_pool = sem("s_pool")
        s_pp = sem("s_pp")    # sigmoid done (+1)

        # python-side tallies for DMA-completion waits
        T = {"dma": 0, "g": 0, "sc": 0, "cc": 0}
        M = {}                   # named checkpoints -> tally value

        def ap4(base_ap, dims):
            return dataclasses.replace(base_ap, ap=[list(base_ap.ap[0])] +
                                       [list(d) for d in dims])

        def col(name, o=0, n=1):
            return smalls[:, cols_smalls[name] + o:cols_smalls[name] + o + n]

        cols_smalls = {"g0": 0, "be0": 1, "g1": 2, "be1": 3, "g2": 4,
                       "be2": 5, "eps": 6, "bout": 7, "cinv": 8}
        for k in list(cols_smalls):
            cols_smalls[k] = cols_smalls[k]

        HB = [X, Y]

        with nc.Block() as block:

            @block.sync
            def _(sync):
                def dma(out, in_, name=None):
                    # serialize: completions ordered => threshold waits sound
                    sync.wait_ge(s_dma, T["dma"])
                    T["dma"] += 16
                    if name:
                        M[name] = T["dma"]
                    sync.dma_start(out=out, in_=in_).then_inc(s_dma, 16)

                dma(idxs_sb[:, :], idxs_d[:, :])
                dma(dinvb[:, :], cpack_d[:, cols["dinvb"]:cols["dinvb"] + SP])
                dma(wsb[:, :], cpack_d[:, cols["w0"]:cols["w0"] + 3 * 128])
                dma(woutb[:, :], cpack_d[:, cols["woutb"]:cols["woutb"] + 128])
                dma(smalls[:, :], cpack_d[:, cols["smalls"]:cols["smalls"] + 16],
                    "smalls")
                dma(p1h[:, :], cpack_d[:, cols["pool1h"]:
                                       cols["pool1h"] + TT * 64])
                # zrow from memset gbuf
                sync.wait_ge(s_z, 1)
                zs = ACC // 128
                dma(zrow.ap().rearrange("(p a) f -> p (a f)", p=128),
                    gbuf[:, 0:zs * 128], "zrow")
                # scrub internal DRAM so runs are independent of device
                # history (stale NaNs from unrelated programs must not be
                # observable through any read-early window)
                dma(acc_d[:, :], zrow[:, :])
                dma(h3nm[:, :], zrow[:, :])
                dma(cc_in[:, :], zrow[0:SP, :])
                for zi in range(NP // ACC):
                    dma(mfull[zi * ACC:(zi + 1) * ACC, :], zrow[:, :])
                dma(mfull[NP - ACC:NP, :], zrow[:, :])
                dma(ar_in[:, :], zrow[0:128, 0:2])
                dma(ar_out[:, :], zrow[0:128, 0:2])
                dma(ar2_in[:, :], zrow[0:G, :])
                dma(ar2_out[:, :], zrow[0:G, :], "scrub")
                for rep in range(reps):
                    dma(X[:, :], xsT_d[:, :], f"x{rep}")
                    for l in range(3):
                        LG = rep * 3 + l
                        Hs, Ms = HB[l % 2], HB[(l + 1) % 2]
                        # M' store (transposing) after all evacs of layer
                        sync.wait_ge(s_ev, LG * len(EG) + len(EG))
                        if flat_dma:
                            dma(cc_in.ap().rearrange("(p t) f -> p (t f)",
                                                     p=128),
                                Ms[:, :], f"st{LG}")
                        else:
                            dma(cc_in.ap().rearrange("(n o) f -> f (n o)",
                                                     o=1),
                                Ms[:, :], f"st{LG}")
                        # zero acc (before scatters; after prior readback)
                        dma(acc_d[:, :], zrow[:, :], f"z{LG}")
                        # readback acc -> Ms (transposing) after scatters
                        sync.wait_ge(s_sc, 16 * sum(
                            1 for _ in _iter_scat(chunks, LG + 1)))
                        if flat_dma:
                            dma(Ms.ap(), acc_d.ap().rearrange(
                                "(p t) f -> p (t f)", p=128)[:, 0:SP],
                                f"rb{LG}")
                        else:
                            dma(Ms.ap(),
                                dataclasses.replace(
                                    acc_d.ap(),
                                    ap=[[1, 128], [H, SP]]),
                                f"rb{LG}")
                        # stats store / load around AllReduce
                        sync.wait_ge(s_u, LG + 1)
                        sync.wait_ge(s_sq, LG + 1)
                        dma(ar_in[:, :], stp[:, :], f"ss{LG}")
                        sync.wait_ge(s_cc, (rep * 7 + 2 * l + 2) *
                                     (16 if no_cc else 1))
                        dma(sta[:, :], ar_out[:, :], f"sl{LG}")
                    # pooling: transposing store of h3 (Y) to node rows
                    sync.wait_ge(s_h, rep * 3 + 3)
                    if flat_dma:
                        dma(h3nm.ap().rearrange("(p t) f -> p (t f)",
                                                p=128)[:, 0:SP],
                            HB[1][:, :], f"h3{rep}")
                    else:
                        dma(h3nm.ap().rearrange("(n o) f -> f (n o)", o=1)[
                            :, 0:SP], HB[1][:, :], f"h3{rep}")
                    # node-major reload: sbuf[p, t, f] = h3nm[t*128+p, f]
                    dma(gbuf[:, 0:TT * 128].rearrange(
                            "p (t f) -> p t f", f=128),
                        dataclasses.replace(
                            h3nm.ap(),
                            ap=[[H, 128], [128 * H, TT], [1, H]]),
                        f"pm{rep}")
                    # pooled partial AllReduce
                    sync.wait_ge(s_pp, rep + 1)
                    dma(ar2_in.ap().rearrange("(p o) f -> p (o f)", p=G),
                        parb[:, :], f"ps{rep}")
                    sync.wait_ge(s_cc, (rep * 7 + 7) * (16 if no_cc else 1))
                    dma(parb[:, :],
                        ar2_out.ap().rearrange("(p o) f -> p (o f)", p=G),
                        f"pl{rep}")
                    sync.wait_ge(s_sig, rep + 1)
                    dma(out_d[:, :], outs[:, :])

            def _cc(gpsimd, kind, op, ins, outs_):
                if no_cc:
                    T["cc"] += 16
                    return gpsimd.dma_start(
                        out=outs_[0].tensor[0:ins[0].shape[0], :],
                        in_=ins[0]).then_inc(s_cc, 16)
                T["cc"] += 1
                return gpsimd.collective_compute(
                    kind, op, replica_groups=rg, ins=ins,
                    outs=outs_).then_inc(s_cc, 1)

            @block.gpsimd
            def _(gpsimd):
                gpsimd.load_library(_mlp_lib)
                red_n = 0
                _regs = {}

                def nreg(w):
                    if w not in _regs:
                        _regs[w] = gpsimd.to_reg(w)
                    return _regs[w]
                for rep in range(reps):
                    for l in range(3):
                        LG = rep * 3 + l
                        gpsimd.wait_ge(s_dma, M[f"st{LG}"])
                        _cc(gpsimd, "AllGather", ALU.bypass,
                            [cc_in[:, :]], [mfull[:, :]])
                        gpsimd.wait_ge(s_cc, T["cc"])
                        gpsimd.wait_ge(s_dma, M[f"z{LG}"])
                        if l == 0 and rep > 0:
                            # pool matmuls of rep-1 still read gbuf
                            gpsimd.wait_ge(s_pool, rep)
                        gsl = LG % 4
                        for (K, nb, calls, _sidx), so in zip(chunks, sc_offs):
                            # gathers overwrite gbuf: wait prior reduce
                            gpsimd.wait_ge(s_red, red_n)
                            off = 0
                            for (bi, w) in calls:
                                if no_gather:
                                    continue
                                o, half = offs[bi]
                                base = mfull[0:HALF, :] if half == 0 else \
                                    mfull[HALF:NP, :]
                                T["g" + str(gsl)] = T.get("g" + str(gsl),
                                                          0) + 16
                                gpsimd.dma_gather(
                                    gbuf[:, off:off + w].rearrange(
                                        "p (c e) -> p c e", e=128),
                                    base, idxs_sb[:, o:o + w // 16],
                                    w, nreg(w), 128,
                                ).then_inc(s_g4[gsl], 16)
                                off += w
                            red_n += 1
                            gpsimd.wait_ge(s_red, red_n)
                            T["sc"] += 16
                            gpsimd.dma_scatter_add(
                                acc_d[:, :],
                                redo[:, 0:nb].rearrange(
                                    "p (c e) -> p c e", e=128),
                                idxs_sb[:, so:so + nb // 16],
                                nb, nreg(nb), 128,
                            ).then_inc(s_sc, 16)
                        gpsimd.wait_ge(s_dma, M[f"ss{LG}"])
                        _cc(gpsimd, "AllReduce", ALU.add,
                            [ar_in[:, :]], [ar_out[:, :]])
                    # pooled partial AllReduce
                    gpsimd.wait_ge(s_dma, M[f"ps{rep}"])
                    _cc(gpsimd, "AllReduce", ALU.add,
                        [ar2_in[:, :]], [ar2_out[:, :]])

            @block.tensor
            def _(tensor):
                for rep in range(reps):
                    for l in range(3):
                        LG = rep * 3 + l
                        Hs = HB[l % 2]
                        for m in range(MM):
                            if m == 0:
                                if l == 0:
                                    tensor.wait_ge(s_dma, M[f"x{rep}"])
                                    tensor.wait_ge(s_dma, M["smalls"])
                                else:
                                    tensor.wait_ge(s_h, LG)
                            if m == 8:
                                tensor.wait_ge(s_ev, LG * len(EG) + 1)
                            w = min(512, SP - m * 512)
                            tensor.matmul(
                                psT[:, m % 8, 0:w],
                                wsb[:, l * 128:(l + 1) * 128],
                                Hs[:, m * 512:m * 512 + w],
                            ).then_inc(s_pet, 1)
                    # pooling: pooled[g, f] = sum_t onehot_t.T @ h3_t
                    tensor.wait_ge(s_dma, M[f"pm{rep}"])
                    for t in range(TT):
                        tensor.matmul(
                            psT[0:G, 5, 0:128],
                            p1h[:, t * 64:t * 64 + 64][:, 0:G],
                            gbuf[:, 0:TT * 128].rearrange(
                                "p (t2 f) -> p t2 f", f=128)[:, t, :],
                            start=(t == 0), stop=(t == TT - 1),
                        )
                    tensor.drain()
                    tensor.matmul(
                        psT[0:1, 6, 0:1], p1h[0:1, 0:1], p1h[0:1, 0:1],
                    ).then_inc(s_pool, 1)

            @block.vector
            def _(vector):
                vector.memset(gbuf[:, 0:(ACC // 128) * 128], 0.0).then_inc(
                    s_z, 1)
                g_n = [0, 0, 0, 0]
                sc_n = 0
                red_n = 0
                for rep in range(reps):
                    for l in range(3):
                        LG = rep * 3 + l
                        Ms = HB[(l + 1) % 2]
                        Hs = HB[l % 2]
                        # psum evacuations with dinv fold
                        for gi, (m0, m1) in enumerate(EG):
                            vector.wait_ge(s_pet, LG * MM + m1)
                            c0, c1 = m0 * 512, min(m1 * 512, SP)
                            vector.scalar_tensor_tensor(
                                Ms[:, c0:c1],
                                psT[:, :, :].rearrange(
                                    "p a b -> p (a b)")[:, c0 - m0 * 512 +
                                                        (m0 % 8) * 512:
                                                        (m0 % 8) * 512 +
                                                        c1 - m0 * 512],
                                1.0,
                                dinvb[:, c0:c1],
                                op0=ALU.mult, op1=ALU.mult,
                            ).then_inc(s_ev, 1)
                        # chunk reduces
                        gsl = LG % 4
                        for (K, nb, calls, _sidx) in chunks:
                            if not no_gather:
                                g_n[gsl] += 16 * len(calls)
                            vector.wait_ge(s_g4[gsl], g_n[gsl])
                            if sc_n:
                                vector.wait_ge(s_sc, sc_n)
                            inap = ap4(gbuf[:, 0:128],
                                       [[128, nb // 128], [1, 128],
                                        [nb, 2 * K]])
                            outap = ap4(redo[:, 0:128],
                                        [[128, nb // 128], [1, 128]])
                            vector.tensor_reduce(
                                outap, inap, mybir.AxisListType.X, ALU.add,
                            ).then_inc(s_red, 1)
                            red_n += 1
                            sc_n += 16
                        # u = dinv * acc with accum
                        vector.wait_ge(s_dma, M[f"rb{LG}"])
                        vector.scalar_tensor_tensor(
                            Hs[:, :], Ms[:, :], 1.0, dinvb[:, :],
                            op0=ALU.mult, op1=ALU.mult,
                            accum_out=stp[:, 0:1],
                        ).then_inc(s_u, 1)
                        # BN stats tail
                        vector.wait_ge(s_dma, M[f"sl{LG}"])
                        vector.tensor_scalar_mul(bnp[:, 0:2], sta[:, 0:2],
                                                 1.0 / meta["N"])
                        vector.drain()
                        vector.tensor_mul(bnp[:, 2:3], bnp[:, 0:1],
                                          bnp[:, 0:1])
                        vector.drain()
                        vector.tensor_sub(bnp[:, 2:3], bnp[:, 1:2],
                                          bnp[:, 2:3])
                        vector.drain()
                        vector.tensor_scalar_add(bnp[:, 2:3], bnp[:, 2:3],
                                                 EPS)
                        vector.drain()
                        vector.reciprocal(bnp[:, 4:5],
                                          bnp[:, 2:3]).then_inc(s_b1, 1)
                        vector.wait_ge(s_b2, LG + 1)
                        vector.tensor_sub(bnp[:, 3:4],
                                          col(f"be{l}"),
                                          bnp[:, 7:8]).then_inc(s_b3, 1)
                    # pooled partial psum -> sbuf
                    vector.wait_ge(s_pool, rep + 1)
                    vector.tensor_copy(parb[:, :],
                                       psT[0:G, 5, 0:128]).then_inc(s_pp, 1)
                    # readout dot
                    vector.wait_ge(s_dma, M[f"pl{rep}"])
                    vector.scalar_tensor_tensor(
                        redo[0:G, 1024:1024 + 128], parb[:, :], 1.0,
                        woutb[0:G, :], op0=ALU.mult, op1=ALU.mult,
                        accum_out=dota[0:G, 0:1],
                    ).then_inc(s_dot, 1)

            @block.scalar
            def _(scalar):
                for rep in range(reps):
                    for l in range(3):
                        LG = rep * 3 + l
                        Hs = HB[l % 2]
                        Ms = HB[(l + 1) % 2]
                        scalar.wait_ge(s_u, LG + 1)
                        scalar.activation(
                            gbuf[:, 0:SP], Hs[:, :], AF.Square,
                            accum_out=stp[:, 1:2],
                        ).then_inc(s_sq, 1)
                        scalar.wait_ge(s_b1, LG + 1)
                        scalar.activation(bnp[:, 5:6], bnp[:, 4:5], AF.Sqrt)
                        scalar.drain()
                        scalar.activation(bnp[:, 6:7], bnp[:, 5:6], AF.Copy,
                                          scale=col(f"g{l}"))
                        scalar.drain()
                        scalar.activation(bnp[:, 7:8], bnp[:, 6:7], AF.Copy,
                                          scale=bnp[:, 0:1]).then_inc(s_b2, 1)
                        scalar.wait_ge(s_b3, LG + 1)
                        scalar.activation(
                            Ms[:, :], Hs[:, :], AF.Relu,
                            bias=bnp[:, 3:4], scale=bnp[:, 6:7],
                        ).then_inc(s_h, 1)
                    scalar.wait_ge(s_dot, rep + 1)
                    scalar.activation(
                        outs[:, :], dota[0:G, 0:1], AF.Sigmoid,
                        bias=col("bout").tensor[0:G,
                                                cols_smalls["bout"]:
                                                cols_smalls["bout"] + 1],
                        scale=col("cinv").tensor[0:G,
                                                 cols_smalls["cinv"]:
                                                 cols_smalls["cinv"] + 1],
                    ).then_inc(s_sig, 1)

        nc.compile()
    return nc


def _iter_scat(chunks, nlayers):
    for _ in range(nlayers):
        for ch in chunks:
            yield ch


def _redtotal(chunks, nlayers):
    return nlayers * len(chunks)


# ---------------------------------------------------------------------------
# entry point
# ---------------------------------------------------------------------------

def kernel(**inputs):
    x = np.asarray(inputs["x"], np.float32)
    edge_index = np.asarray(inputs["edge_index"])
    batch = np.asarray(inputs["batch"])
    G = 64
    P = P_CORES

    meta, per_core = _prep(x, edge_index, batch, P, G)
    in_maps = []
    cols = CK = None
    for c in range(P):
        cpack, cols, CK = _pack_cpack(meta, per_core[c], inputs)
        in_maps.append({
            "cpack": cpack,
            "xsT": per_core[c]["xsT"],
            "idxs": per_core[c]["idxs"],
        })

    nc = build_nc(meta, per_core[0], cols, CK, reps=1)

    if os.environ.get("GCN_SIM"):
        from concourse import bass_interp
        sim = bass_interp.MultiCoreSim(nc, P)
        for c in range(P):
            for k, v in in_maps[c].items():
                sim.cores[c].tensor(k)[:] = v
        sim.simulate()
        return np.asarray(sim.cores[0].mem_tensor("out"), np.float32)

    from concourse.bass_utils import run_bass_kernel_spmd
    res = run_bass_kernel_spmd(nc, in_maps, core_ids=list(range(P)))
    return np.asarray(res.results[0]["out"], np.float32)
